# revision 4
# baseline (speedup 1.0000x reference)
"""MinGRU Trainium2 kernel.

Computation (per batch element b):
    z = sigmoid(X @ Wz + bz)          X: [T, DIN], Wz: [DIN, D]
    n = tanh(X @ Wn + bn)
    a = z * (1 - mask)[:, None]
    bb = (1 - z) * n
    h_t = a_t * h_{t-1} + bb_t        (affine scan over time, h_0 = initial_carry)
    returns (h [B, T, D], h[:, -1, :])

Strategy:
  - Data-parallel over batch: 8 batch elements -> 8 NeuronCores, no collectives.
  - Host pre-transposes X to X^T [DIN, T] (bf16) so the matmul needs no
    on-device transpose and produces outputs in [d, t] layout directly
    (lhsT = W [i, d] slice, rhs = X^T [i, t] slice).
  - [d, t] layout puts time on the free axis: the whole recurrence is done by
    the DVE tensor_tensor_scan instruction (state = a*state - (z-1)*n).
  - b is computed as nb = (z-1)*n in one fused scalar_tensor_tensor op and the
    scan uses op1=subtract, so h = a*h - nb = a*h + (1-z)*n.
  - The mask complement is pre-broadcast on host to [128, T] so a = z*mc is a
    single DVE multiply.
  - Output h^T [D, T] f32 is written to DRAM; host transposes back.
"""

import numpy as np
import ml_dtypes

BF16 = ml_dtypes.bfloat16

B, T, DIN, D = 8, 4096, 1024, 1024
P = 128
NT = 512  # time chunk = psum free dim


def _build_nc(T=T, DIN=DIN, D=D, NT=NT):
    from contextlib import ExitStack

    import concourse.bacc as bacc
    import concourse.tile as tile
    from concourse import mybir

    f32 = mybir.dt.float32
    bf16 = mybir.dt.bfloat16
    Alu = mybir.AluOpType
    Act = mybir.ActivationFunctionType

    KT = DIN // P  # k tiles (contraction)
    MT = D // P    # d tiles (output partition blocks)
    JT = T // NT   # time chunks

    nc = bacc.Bacc("TRN2", target_bir_lowering=False)

    xT = nc.declare_dram_parameter("xT", [DIN, T], bf16, isOutput=False)
    wz = nc.declare_dram_parameter("wz", [DIN, D], bf16, isOutput=False)
    wn = nc.declare_dram_parameter("wn", [DIN, D], bf16, isOutput=False)
    bzp = nc.declare_dram_parameter("bzp", [P, MT], f32, isOutput=False)
    bnp = nc.declare_dram_parameter("bnp", [P, MT], f32, isOutput=False)
    mc = nc.declare_dram_parameter("mc", [P, T], f32, isOutput=False)
    h0 = nc.declare_dram_parameter("h0", [P, MT], f32, isOutput=False)
    hT = nc.declare_dram_parameter("hT", [D, T], f32, isOutput=True)

    with ExitStack() as ctx:
        tc = ctx.enter_context(tile.TileContext(nc))
        wpool = ctx.enter_context(tc.tile_pool(name="w", bufs=1))
        cpool = ctx.enter_context(tc.tile_pool(name="c", bufs=1))
        xpool = ctx.enter_context(tc.tile_pool(name="x", bufs=2))
        spool = ctx.enter_context(tc.tile_pool(name="s", bufs=3))
        hpool = ctx.enter_context(tc.tile_pool(name="h", bufs=2))
        ppool = ctx.enter_context(tc.tile_pool(name="p", bufs=2, space="PSUM"))

        # Persistent: weights (as [128k, D] stripes), mask complement, biases, h0.
        wz_sb, wn_sb = [], []
        for k in range(KT):
            tz = wpool.tile([P, D], bf16, name=f"wzsb{k}", tag=f"wz{k}")
            nc.sync.dma_start(tz[:], wz[k * P:(k + 1) * P, :])
            wz_sb.append(tz)
            tn = wpool.tile([P, D], bf16, name=f"wnsb{k}", tag=f"wn{k}")
            nc.sync.dma_start(tn[:], wn[k * P:(k + 1) * P, :])
            wn_sb.append(tn)
        mc_sb = cpool.tile([P, T], f32, name="mcsb")
        nc.sync.dma_start(mc_sb[:], mc[:, :])
        bz_sb = cpool.tile([P, MT], f32, name="bzsb")
        nc.sync.dma_start(bz_sb[:], bzp[:, :])
        bn_sb = cpool.tile([P, MT], f32, name="bnsb")
        nc.sync.dma_start(bn_sb[:], bnp[:, :])
        h0_sb = cpool.tile([P, MT], f32, name="h0sb")
        nc.sync.dma_start(h0_sb[:], h0[:, :])

        prev_h = [None] * MT
        for j in range(JT):
            xk = []
            for k in range(KT):
                tx = xpool.tile([P, NT], bf16, name=f"xk{k}", tag=f"xk{k}")
                nc.sync.dma_start(tx[:], xT[k * P:(k + 1) * P, j * NT:(j + 1) * NT])
                xk.append(tx)
            for m in range(MT):
                pz = ppool.tile([P, NT], f32, name="pz", tag="pz")
                pn = ppool.tile([P, NT], f32, name="pn", tag="pn")
                for k in range(KT):
                    nc.tensor.matmul(
                        pz[:], wz_sb[k][:, m * P:(m + 1) * P], xk[k][:],
                        start=(k == 0), stop=(k == KT - 1),
                    )
                for k in range(KT):
                    nc.tensor.matmul(
                        pn[:], wn_sb[k][:, m * P:(m + 1) * P], xk[k][:],
                        start=(k == 0), stop=(k == KT - 1),
                    )
                zt = spool.tile([P, NT], f32, name="zt", tag="zt")
                nc.scalar.activation(zt[:], pz[:], Act.Sigmoid, bias=bz_sb[:, m:m + 1])
                nt_ = spool.tile([P, NT], f32, name="nt_", tag="nt_")
                nc.scalar.activation(nt_[:], pn[:], Act.Tanh, bias=bn_sb[:, m:m + 1])
                # nb = (z - 1) * n  (= -b)
                nb = spool.tile([P, NT], f32, name="nb", tag="nb")
                nc.vector.scalar_tensor_tensor(
                    nb[:], zt[:], 1.0, nt_[:], op0=Alu.subtract, op1=Alu.mult
                )
                # a = z * (1 - mask)
                at = spool.tile([P, NT], f32, name="at", tag="at")
                nc.vector.tensor_tensor(
                    at[:], zt[:], mc_sb[:, j * NT:(j + 1) * NT], op=Alu.mult
                )
                # h = a*h_prev - nb  (scan along time)
                ht = hpool.tile([P, NT], f32, name="ht", tag=f"ht{m}")
                init = h0_sb[:, m:m + 1] if j == 0 else prev_h[m][:, NT - 1:NT]
                nc.vector.tensor_tensor_scan(
                    ht[:], at[:], nb[:], initial=init,
                    op0=Alu.mult, op1=Alu.subtract,
                )
                prev_h[m] = ht
                nc.sync.dma_start(hT[m * P:(m + 1) * P, j * NT:(j + 1) * NT], ht[:])
    nc.finalize()  # runs Bacc.compile(): reg alloc + wait splitting
    return nc


_cached_nc = None
last_results = None  # BassKernelResults of the most recent run (for test.py)


def _host_prep(X, mask, h0, Wz, bz, Wn, bn, T=T, DIN=DIN, D=D):
    """Build per-core input maps (host-side shard/cast/transpose)."""
    MT = D // P
    nb_ = X.shape[0]
    wz_b = np.ascontiguousarray(Wz.astype(BF16))
    wn_b = np.ascontiguousarray(Wn.astype(BF16))
    bz_t = np.ascontiguousarray(bz.astype(np.float32).reshape(MT, P).T)
    bn_t = np.ascontiguousarray(bn.astype(np.float32).reshape(MT, P).T)
    in_maps = []
    for b in range(nb_):
        xT_b = np.ascontiguousarray(X[b].T.astype(BF16))  # [DIN, T]
        mc_b = np.ascontiguousarray(
            np.broadcast_to((1.0 - mask[b]).astype(np.float32), (P, T))
        )
        h0_b = np.ascontiguousarray(h0[b].astype(np.float32).reshape(MT, P).T)
        in_maps.append(
            dict(xT=xT_b, wz=wz_b, wn=wn_b, bzp=bz_t, bnp=bn_t, mc=mc_b, h0=h0_b)
        )
    return in_maps


def kernel(**inputs):
    global _cached_nc, last_results
    from concourse.bass_utils import run_bass_kernel_spmd

    X = np.asarray(inputs["inputs"], dtype=np.float32)
    mask = np.asarray(inputs["mask"])
    h0 = np.asarray(inputs["initial_carry"], dtype=np.float32)
    Wz = np.asarray(inputs["Wz"], dtype=np.float32)
    bz = np.asarray(inputs["bz"], dtype=np.float32)
    Wn = np.asarray(inputs["Wn"], dtype=np.float32)
    bn = np.asarray(inputs["bn"], dtype=np.float32)

    if _cached_nc is None:
        _cached_nc = _build_nc()

    in_maps = _host_prep(X, mask, h0, Wz, bz, Wn, bn)
    res = run_bass_kernel_spmd(_cached_nc, in_maps, core_ids=list(range(B)))
    last_results = res
    hTs = np.stack([res.results[b]["hT"] for b in range(B)])  # [B, D, T] f32
    carry = np.ascontiguousarray(hTs.transpose(0, 2, 1))      # [B, T, D]
    return carry, np.ascontiguousarray(carry[:, -1, :])
